# revision 7
# baseline (speedup 1.0000x reference)
"""Trainium2 Bass kernel for nn_DecoderAttention (LSTM cell + additive attention
+ pointer-generator vocab distribution).

Sharding (8 cores):
  - wg_w (the [50003, 3072] output projection) is row-sharded (vocab-parallel);
    each core streams a [3072, 6272] pre-transposed fp32 shard through the PE
    at full rate (float32r) against the stationary x_catT chunks.
  - The LSTM cell is hidden-sharded: core k computes h_new/c_new for hidden
    dims [128k, 128k+128) for all 32 batches.
  - Attention is batch-sharded: core k handles batches [4k, 4k+4).
  - Two small AllGathers (h_new, context) knit the stages together; the
    weight stream is ordered h_parent-third -> h_new-third -> context-third
    so DMA never waits on the dependency chain.
  - Per-shard log-softmax stats (logsumexp) are computed on device; the final
    cross-shard combine and output concatenation happen on host.
"""

import numpy as np

import concourse.bass as bass
import concourse.bacc as bacc
import concourse.tile as tile
import concourse.mybir as mybir
from concourse.bass_utils import run_bass_kernel_spmd

dt = mybir.dt
F32 = dt.float32
F32R = dt.float32r
AF = mybir.ActivationFunctionType

N_CORES = 8
B, L, H = 32, 128, 1024
EIN = 768           # embN(256) + embT(512)
XH = EIN + H        # 1792 = lstm input + hidden
KC_XH = XH // 128   # 14
V = 50003           # VOUT
VPC = 6272          # padded vocab shard per core (8*6272 = 50176)
BPC = B // N_CORES  # 4 batches per core
HC = 8              # hidden chunks of 128
NEG = -1.0e9        # pad bias for log-softmax masking

# N-strip layout of the vocab shard: 6 strips of 1024 (2 x 512 matmuls each)
# plus one tail strip of 128.
STRIPS = [(g * 1024, 1024, 2) for g in range(6)] + [(6144, 128, 1)]
NCHUNKS = 13  # 12 x 512 + 1 x 128


def _r(ap):
    return ap.bitcast(F32R)


def build_nc(single_core: bool = False):
    """Trace + compile the SPMD program. Returns the compiled Bacc."""
    nc = bacc.Bacc(
        "TRN2",
        target_bir_lowering=False,
        debug=False,
        num_devices=1 if single_core else N_CORES,
    )

    # ---- DRAM I/O (per core) ----
    wgT = nc.dram_tensor("wgT", [3072, VPC], F32R, kind="ExternalInput")
    wgb = nc.dram_tensor("wgb", [1, VPC], F32R, kind="ExternalInput")
    wlstmT = nc.dram_tensor("wlstmT", [XH, 512], F32, kind="ExternalInput")
    lstmb = nc.dram_tensor("lstmb", [128, 4], F32, kind="ExternalInput")
    xhT = nc.dram_tensor("xhT", [XH, 32], F32, kind="ExternalInput")
    c0T = nc.dram_tensor("c0T", [128, 32], F32, kind="ExternalInput")
    WhT = nc.dram_tensor("WhT", [H, H], F32, kind="ExternalInput")
    WmT = nc.dram_tensor("WmT", [H, H], F32, kind="ExternalInput")
    attb = nc.dram_tensor("attb", [128, HC], F32, kind="ExternalInput")
    encT = nc.dram_tensor("encT", [H, BPC * L], F32, kind="ExternalInput")
    encN = nc.dram_tensor("encN", [BPC * L, H], F32, kind="ExternalInput")
    vT = nc.dram_tensor("vT", [128, HC], F32, kind="ExternalInput")
    maskf = nc.dram_tensor("maskf", [1, BPC * L], F32, kind="ExternalInput")
    sel = nc.dram_tensor("sel", [B, BPC], F32, kind="ExternalInput")
    ws1T = nc.dram_tensor("ws1T", [128, HC], F32, kind="ExternalInput")
    ws2T = nc.dram_tensor("ws2T", [128, HC], F32, kind="ExternalInput")
    wsb4 = nc.dram_tensor("wsb4", [BPC, 1], F32, kind="ExternalInput")
    hparT = nc.dram_tensor("hparT", [H, 32], F32R, kind="ExternalInput")
    ident = nc.dram_tensor("ident", [128, 128], F32, kind="ExternalInput")
    ones1 = nc.dram_tensor("ones1", [1, 32], F32R, kind="ExternalInput")

    z_out = nc.dram_tensor("z", [32, VPC], F32, kind="ExternalOutput")
    lse_out = nc.dram_tensor("lse", [32, 1], F32, kind="ExternalOutput")
    hnat_out = nc.dram_tensor("hnat", [32, 128], F32, kind="ExternalOutput")
    cT_out = nc.dram_tensor("cT", [128, 32], F32, kind="ExternalOutput")
    clp_out = nc.dram_tensor("clp", [BPC, L], F32, kind="ExternalOutput")
    st_out = nc.dram_tensor("st", [BPC, 1], F32, kind="ExternalOutput")

    groups = [[0]] if single_core else [list(range(N_CORES))]

    with tile.TileContext(nc) as tc:
        with (
            tc.tile_pool(name="stream", bufs=16) as streamp,
            tc.tile_pool(name="weights", bufs=4) as wxp,
            tc.tile_pool(name="encp", bufs=1) as encp,
            tc.tile_pool(name="encnp", bufs=2) as encnp,
            tc.tile_pool(name="zp", bufs=1) as zp,
            tc.tile_pool(name="smallp", bufs=1) as smallp,
            tc.tile_pool(name="tanhp", bufs=3) as tanhp,
            tc.tile_pool(name="wgbp", bufs=2) as wgbp,
            tc.tile_pool(name="expp", bufs=2) as expp,
            tc.tile_pool(name="zps", bufs=3, space="PSUM") as zpsp,
            tc.tile_pool(name="bigps", bufs=2, space="PSUM") as bigps,
            tc.tile_pool(name="scps", bufs=1, space="PSUM") as scps,
            tc.tile_pool(name="tinyps", bufs=2, space="PSUM") as tinyps,
            tc.tile_pool(name="dram", bufs=1, space="DRAM") as dramp,
        ):
            # ---------- small constant loads ----------
            ident_sb = smallp.tile([128, 128], F32, tag="ident")
            nc.sync.dma_start(ident_sb[:], ident[:])
            ones_sb = smallp.tile([1, 32], F32R, tag="ones1")
            nc.sync.dma_start(ones_sb[:], ones1[:])
            sel_sb = smallp.tile([B, BPC], F32, tag="sel")
            nc.sync.dma_start(sel_sb[:], sel[:])
            lstmb_sb = smallp.tile([128, 4], F32, tag="lstmb")
            nc.sync.dma_start(lstmb_sb[:], lstmb[:])
            attb_sb = smallp.tile([128, HC], F32, tag="attb")
            nc.sync.dma_start(attb_sb[:], attb[:])
            vT_sb = smallp.tile([128, HC], F32, tag="vT")
            nc.sync.dma_start(vT_sb[:], vT[:])
            ws1_sb = smallp.tile([128, HC], F32, tag="ws1")
            nc.sync.dma_start(ws1_sb[:], ws1T[:])
            ws2_sb = smallp.tile([128, HC], F32, tag="ws2")
            nc.sync.dma_start(ws2_sb[:], ws2T[:])
            wsb_sb = smallp.tile([BPC, 1], F32, tag="wsb")
            nc.sync.dma_start(wsb_sb[:], wsb4[:])
            maskf_sb = smallp.tile([1, BPC * L], F32, tag="maskf")
            nc.sync.dma_start(maskf_sb[:], maskf[:])
            xhT_sb = smallp.tile([128, KC_XH, 32], F32, tag="xhT")
            nc.sync.dma_start(
                xhT_sb[:], xhT[:].rearrange("(c p) b -> p c b", p=128)
            )
            c0T_sb = smallp.tile([128, 32], F32, tag="c0T")
            nc.sync.dma_start(c0T_sb[:], c0T[:])

            # x_catT stationary chunks: 24 x [128, 32].
            # 0..7 = context (A), 8..15 = h_new (B), 16..23 = h_parent (C)
            xcat = [
                smallp.tile([128, 32], F32R, tag=f"xcat{kc}", name=f"xcat{kc}")
                for kc in range(24)
            ]
            hparT_r = hparT[:].rearrange("(c p) b -> c p b", p=128)
            for hcc in range(HC):
                nc.sync.dma_start(xcat[16 + hcc][:], hparT_r[hcc])

            # ---------- LSTM (hidden-sharded: this core's 128 h dims) ----------
            gates_ps = tinyps.tile([128, 4, 32], F32, tag="tiny")
            wls = []
            for kc in range(KC_XH):
                wl = streamp.tile([128, 512], F32, tag="wg", name=f"wl{kc}")
                nc.sync.dma_start(
                    wl[:],
                    wlstmT[:].rearrange("(c p) g -> c p g", p=128)[kc],
                )
                wls.append(wl)
            for gc in range(4):
                for kc in range(KC_XH):
                    nc.tensor.matmul(
                        gates_ps[:, gc, :],
                        wls[kc][:, gc * 128:(gc + 1) * 128],
                        xhT_sb[:, kc, :],
                        start=(kc == 0),
                        stop=(kc == KC_XH - 1),
                    )
            ig = smallp.tile([128, 32], F32, tag="ig")
            fg = smallp.tile([128, 32], F32, tag="fg")
            gg = smallp.tile([128, 32], F32, tag="gg")
            og = smallp.tile([128, 32], F32, tag="og")
            nc.scalar.activation(ig[:], gates_ps[:, 0, :], AF.Sigmoid, bias=lstmb_sb[:, 0:1])
            nc.scalar.activation(fg[:], gates_ps[:, 1, :], AF.Sigmoid, bias=lstmb_sb[:, 1:2])
            nc.scalar.activation(gg[:], gates_ps[:, 2, :], AF.Tanh, bias=lstmb_sb[:, 2:3])
            nc.scalar.activation(og[:], gates_ps[:, 3, :], AF.Sigmoid, bias=lstmb_sb[:, 3:4])
            cnew = smallp.tile([128, 32], F32, tag="cnew")
            nc.vector.tensor_mul(cnew[:], fg[:], c0T_sb[:])
            nc.vector.tensor_mul(ig[:], ig[:], gg[:])
            nc.vector.tensor_add(cnew[:], cnew[:], ig[:])
            nc.sync.dma_start(cT_out[:], cnew[:])
            hnewT = smallp.tile([128, 32], F32, tag="hnewT")
            nc.scalar.activation(hnewT[:], cnew[:], AF.Tanh)
            nc.vector.tensor_mul(hnewT[:], hnewT[:], og[:])

            # natural layout [32, 128] for the gather
            hn_ps = tinyps.tile([32, 128], F32, tag="tiny")
            nc.tensor.transpose(hn_ps[:], hnewT[:], ident_sb[:])
            hnat = smallp.tile([32, 128], F32, tag="hnat")
            nc.vector.tensor_copy(hnat[:], hn_ps[:])
            nc.sync.dma_start(hnat_out[:], hnat[:])

            # ---------- AllGather 1: h_new ----------
            g1in = dramp.tile([32, 128], F32, tag="g1in")
            g1out = dramp.tile([N_CORES * 32, 128], F32, tag="g1out")
            nc.sync.dma_start(g1in[:], hnat[:])
            if single_core:
                for c in range(N_CORES):
                    nc.sync.dma_start(g1out[c * 32:(c + 1) * 32, :], g1in[:])
            else:
                nc.gpsimd.collective_compute(
                    "AllGather",
                    mybir.AluOpType.bypass,
                    replica_groups=groups,
                    ins=[g1in[:]],
                    outs=[g1out[:]],
                )
            # full h_new, natural layout: [32 batches, (hc, 128)]
            hna = smallp.tile([32, HC, 128], F32, tag="hna")
            nc.sync.dma_start(
                hna[:],
                g1out[:].rearrange("(c b) h -> b c h", b=32),
            )

            # x_catT B part: transpose each [32, 128] slice -> [128, 32]
            for hcc in range(HC):
                tp = tinyps.tile([128, 32], F32, tag="tiny")
                nc.tensor.transpose(tp[:], hna[:, hcc, :], ident_sb[:32, :32])
                nc.vector.tensor_copy(xcat[8 + hcc][:], tp[:])

            # own-batch h in T layout: h_ownT[h, j] for this core's 4 batches
            hown = smallp.tile([128, HC, BPC], F32, tag="hown")
            for hcc in range(HC):
                ho_ps = tinyps.tile([128, BPC], F32, tag="tiny")
                nc.tensor.matmul(ho_ps[:], hna[:, hcc, :], sel_sb[:])
                nc.vector.tensor_copy(hown[:, hcc, :], ho_ps[:])

            # ---------- attention (batch-sharded: this core's 4 batches) ----------
            encT_sb = encp.tile([128, HC, BPC * L], F32, tag="encT")
            nc.sync.dma_start(
                encT_sb[:], encT[:].rearrange("(c p) n -> p c n", p=128)
            )

            # preh[h', b] = sum_h WhT[h, h'] * h_own[h, b]  (+ Wh_b + Wm_b)
            prehb = smallp.tile([128, HC, BPC], F32, tag="prehb")
            WhT_r = WhT[:].rearrange("(ci p) o -> p ci o", p=128)
            for hco in range(HC):
                whb = wxp.tile([128, HC, 128], F32, tag="wx")
                nc.sync.dma_start(
                    whb[:], WhT_r[:, :, hco * 128:(hco + 1) * 128]
                )
                ph_ps = tinyps.tile([128, BPC], F32, tag="tiny")
                for hci in range(HC):
                    nc.tensor.matmul(
                        ph_ps[:],
                        whb[:, hci, :],
                        hown[:, hci, :],
                        start=(hci == 0),
                        stop=(hci == HC - 1),
                    )
                nc.scalar.activation(
                    prehb[:, hco, :], ph_ps[:], AF.Identity,
                    bias=attb_sb[:, hco:hco + 1],
                )

            # prem + tanh + scores
            sc_ps = scps.tile([1, BPC * L], F32, tag="sc")
            WmT_r = WmT[:].rearrange("(ci p) o -> p ci o", p=128)
            for hco in range(HC):
                wmb = wxp.tile([128, HC, 128], F32, tag="wx")
                nc.sync.dma_start(
                    wmb[:], WmT_r[:, :, hco * 128:(hco + 1) * 128]
                )
                pm_ps = bigps.tile([128, BPC * L], F32, tag="big")
                for hci in range(HC):
                    nc.tensor.matmul(
                        pm_ps[:],
                        wmb[:, hci, :],
                        encT_sb[:, hci, :],
                        start=(hci == 0),
                        stop=(hci == HC - 1),
                    )
                th = tanhp.tile([128, BPC * L], F32, tag="tanh")
                for b in range(BPC):
                    nc.scalar.activation(
                        th[:, b * L:(b + 1) * L],
                        pm_ps[:, b * L:(b + 1) * L],
                        AF.Tanh,
                        bias=prehb[:, hco, b:b + 1],
                    )
                nc.tensor.matmul(
                    sc_ps[:],
                    vT_sb[:, hco:hco + 1],
                    th[:],
                    start=(hco == 0),
                    stop=(hco == HC - 1),
                )

            scores = smallp.tile([1, BPC * L], F32, tag="scores")
            nc.vector.tensor_add(scores[:], sc_ps[:], maskf_sb[:])
            sc4 = smallp.tile([BPC, L], F32, tag="sc4")
            nc.sync.dma_start(sc4[:], scores[:])

            # softmax / log-softmax over L per batch
            m4n = smallp.tile([BPC, 1], F32, tag="m4n")
            nc.vector.reduce_max(m4n[:], sc4[:], axis=mybir.AxisListType.X, negate=True)
            exp4 = smallp.tile([BPC, L], F32, tag="exp4")
            sum4 = smallp.tile([BPC, 1], F32, tag="sum4")
            nc.scalar.activation(exp4[:], sc4[:], AF.Exp, bias=m4n[:], accum_out=sum4[:])
            rec4 = smallp.tile([BPC, 1], F32, tag="rec4")
            nc.vector.reciprocal(rec4[:], sum4[:])
            attn4 = smallp.tile([BPC, L], F32, tag="attn4")
            nc.vector.tensor_scalar_mul(attn4[:], exp4[:], rec4[:])
            lns4 = smallp.tile([BPC, 1], F32, tag="lns4")
            nc.scalar.activation(lns4[:], sum4[:], AF.Ln)

            at_ps = tinyps.tile([128, BPC], F32, tag="tiny")
            nc.tensor.transpose(at_ps[:], attn4[:], ident_sb[:BPC, :BPC])
            attnT = smallp.tile([128, BPC], F32, tag="attnT")
            nc.vector.tensor_copy(attnT[:], at_ps[:])

            # context in T layout: ctxT[h, b] = sum_l encN[b][l, h] * attnT[l, b]
            ctxT = smallp.tile([128, HC, BPC], F32, tag="ctxT")
            for b in range(BPC):
                en = encnp.tile([128, H], F32, tag="encN")
                nc.sync.dma_start(en[:], encN[b * L:(b + 1) * L, :])
                cx_ps = tinyps.tile([128, HC, 1], F32, tag="tiny")
                for hcc in range(HC):
                    nc.tensor.matmul(
                        cx_ps[:, hcc, :],
                        en[:, hcc * 128:(hcc + 1) * 128],
                        attnT[:, b:b + 1],
                    )
                nc.vector.tensor_copy(ctxT[:, :, b:b + 1], cx_ps[:])

            # context natural layout for the gather: [4, 1024]
            ctxn = smallp.tile([BPC, H], F32, tag="ctxn")
            for hcc in range(HC):
                cn_ps = tinyps.tile([BPC, 128], F32, tag="tiny")
                nc.tensor.transpose(cn_ps[:], ctxT[:, hcc, :], ident_sb[:])
                nc.vector.tensor_copy(ctxn[:, hcc * 128:(hcc + 1) * 128], cn_ps[:])

            # ---------- AllGather 2: context ----------
            g2in = dramp.tile([BPC, H], F32, tag="g2in")
            g2out = dramp.tile([B, H], F32, tag="g2out")
            nc.sync.dma_start(g2in[:], ctxn[:])
            if single_core:
                for c in range(N_CORES):
                    nc.sync.dma_start(g2out[c * BPC:(c + 1) * BPC, :], g2in[:])
            else:
                nc.gpsimd.collective_compute(
                    "AllGather",
                    mybir.AluOpType.bypass,
                    replica_groups=groups,
                    ins=[g2in[:]],
                    outs=[g2out[:]],
                )
            ctxa = smallp.tile([B, H], F32, tag="ctxa")
            nc.sync.dma_start(ctxa[:], g2out[:])
            for hcc in range(HC):
                tp = tinyps.tile([128, 32], F32, tag="tiny")
                nc.tensor.transpose(
                    tp[:], ctxa[:, hcc * 128:(hcc + 1) * 128], ident_sb[:32, :32]
                )
                nc.vector.tensor_copy(xcat[hcc][:], tp[:])

            # ---------- pointer switch s_t (this core's 4 batches) ----------
            p_ps = tinyps.tile([1, BPC], F32, tag="tiny")
            for hcc in range(HC):
                nc.tensor.matmul(
                    p_ps[:], ws1_sb[:, hcc:hcc + 1], ctxT[:, hcc, :],
                    start=(hcc == 0), stop=False,
                )
            for hcc in range(HC):
                nc.tensor.matmul(
                    p_ps[:], ws2_sb[:, hcc:hcc + 1], hown[:, hcc, :],
                    start=False, stop=(hcc == HC - 1),
                )
            p1 = smallp.tile([1, BPC], F32, tag="p1")
            nc.vector.tensor_copy(p1[:], p_ps[:])
            p4 = smallp.tile([BPC, 1], F32, tag="p4")
            nc.sync.dma_start(p4[:], p1[:])
            nc.vector.tensor_add(p4[:], p4[:], wsb_sb[:])
            # s_t = log sigmoid(p) = -softplus(-p) = -ln(1 + exp(-p))
            e_n = smallp.tile([BPC, 1], F32, tag="e_n")
            nc.scalar.activation(e_n[:], p4[:], AF.Exp, scale=-1.0)
            nc.vector.tensor_scalar_add(e_n[:], e_n[:], 1.0)
            sp_n = smallp.tile([BPC, 1], F32, tag="sp_n")
            nc.scalar.activation(sp_n[:], e_n[:], AF.Ln)
            st4 = smallp.tile([BPC, 1], F32, tag="st4")
            nc.vector.tensor_scalar_mul(st4[:], sp_n[:], -1.0)
            nc.sync.dma_start(st_out[:], st4[:])
            # log(1 - sigmoid(p)) = -softplus(p) = -ln(1 + exp(p))
            e_p = smallp.tile([BPC, 1], F32, tag="e_p")
            nc.scalar.activation(e_p[:], p4[:], AF.Exp)
            nc.vector.tensor_scalar_add(e_p[:], e_p[:], 1.0)
            sp_p = smallp.tile([BPC, 1], F32, tag="sp_p")
            nc.scalar.activation(sp_p[:], e_p[:], AF.Ln)
            # copy_logp = log_attn + logsig(-p)
            #           = sc4 - max - ln(sum) - softplus(p)
            shift = smallp.tile([BPC, 1], F32, tag="shift")
            nc.vector.tensor_add(shift[:], lns4[:], sp_p[:])
            nc.vector.tensor_sub(shift[:], m4n[:], shift[:])
            clp = smallp.tile([BPC, L], F32, tag="clp")
            nc.vector.tensor_scalar_add(clp[:], sc4[:], shift[:])
            nc.sync.dma_start(clp_out[:], clp[:])

            # ---------- big vocab matmul: 3 K-phases over the shard ----------
            z_tiles = []
            for j in range(NCHUNKS):
                n = 512 if j < 12 else 128
                z_tiles.append(zp.tile([32, n], F32, tag=f"z{j}", name=f"zt{j}"))

            phases = [
                ("C", list(range(16, 24))),
                ("B", list(range(8, 16))),
                ("A", list(range(0, 8))),
            ]
            for pname, kcs in phases:
                for (off, w, nsub) in STRIPS:
                    strips = {}
                    for kc in kcs:
                        s = streamp.tile([128, w], F32R, tag="wg")
                        nc.sync.dma_start(
                            s[:], wgT[kc * 128:(kc + 1) * 128, off:off + w]
                        )
                        strips[kc] = s
                    for sidx in range(nsub):
                        j = off // 512 + sidx
                        n = 512 if j < 12 else 128
                        so = sidx * 512
                        ps = zpsp.tile([32, n], F32, tag="zps")
                        for i, kc in enumerate(kcs):
                            nc.tensor.matmul(
                                ps[:],
                                xcat[kc][:],
                                strips[kc][:, so:so + n],
                                start=(i == 0),
                                stop=(i == len(kcs) - 1 and pname != "C"),
                            )
                        if pname == "C":
                            wgb_t = wgbp.tile([1, n], F32R, tag="wgb")
                            nc.sync.dma_start(wgb_t[:], wgb[0:1, off + so:off + so + n])
                            nc.tensor.matmul(
                                ps[:], ones_sb[:], wgb_t[:],
                                start=False, stop=True,
                            )
                            nc.vector.tensor_copy(z_tiles[j][:], ps[:])
                        else:
                            nc.vector.tensor_add(z_tiles[j][:], z_tiles[j][:], ps[:])

            # ---------- shard-local logsumexp + output ----------
            mparts = smallp.tile([32, NCHUNKS], F32, tag="mparts")
            for j in range(NCHUNKS):
                nc.vector.reduce_max(
                    mparts[:, j:j + 1], z_tiles[j][:], axis=mybir.AxisListType.X
                )
            mneg = smallp.tile([32, 1], F32, tag="mneg")
            nc.vector.reduce_max(
                mneg[:], mparts[:], axis=mybir.AxisListType.X, negate=True
            )
            sparts = smallp.tile([32, NCHUNKS], F32, tag="sparts")
            for j in range(NCHUNKS):
                n = 512 if j < 12 else 128
                scr = expp.tile([32, 512], F32, tag="exps")
                nc.scalar.activation(
                    scr[:, :n], z_tiles[j][:], AF.Exp,
                    bias=mneg[:], accum_out=sparts[:, j:j + 1],
                )
                nc.sync.dma_start(z_out[:, j * 512:j * 512 + n], z_tiles[j][:])
            ssum = smallp.tile([32, 1], F32, tag="ssum")
            nc.vector.reduce_sum(ssum[:], sparts[:], axis=mybir.AxisListType.X)
            lns = smallp.tile([32, 1], F32, tag="lns")
            nc.scalar.activation(lns[:], ssum[:], AF.Ln)
            lse = smallp.tile([32, 1], F32, tag="lse")
            nc.vector.tensor_sub(lse[:], lns[:], mneg[:])
            nc.sync.dma_start(lse_out[:], lse[:])

    nc.compile()
    return nc


# ----------------------------------------------------------------------------
# host-side sharding / unsharding
# ----------------------------------------------------------------------------

def make_in_maps(inputs):
    f32 = np.float32
    n_input = np.asarray(inputs["n_input"]).astype(np.int64)
    t_input = np.asarray(inputs["t_input"]).astype(np.int64)
    h0 = np.asarray(inputs["h0"], dtype=f32)[0]        # [32, 1024]
    c0 = np.asarray(inputs["c0"], dtype=f32)[0]
    enc = np.asarray(inputs["enc_out"], dtype=f32)     # [32, 128, 1024]
    mask = np.asarray(inputs["mask"])
    h_parent = np.asarray(inputs["h_parent"], dtype=f32)
    embN = np.asarray(inputs["embN"], dtype=f32)
    embT = np.asarray(inputs["embT"], dtype=f32)
    W_ih = np.asarray(inputs["W_ih"], dtype=f32)
    W_hh = np.asarray(inputs["W_hh"], dtype=f32)
    b_ih = np.asarray(inputs["b_ih"], dtype=f32)
    b_hh = np.asarray(inputs["b_hh"], dtype=f32)
    Wh_w = np.asarray(inputs["Wh_w"], dtype=f32)
    Wh_b = np.asarray(inputs["Wh_b"], dtype=f32)
    Wm_w = np.asarray(inputs["Wm_w"], dtype=f32)
    Wm_b = np.asarray(inputs["Wm_b"], dtype=f32)
    v_w = np.asarray(inputs["v_w"], dtype=f32)
    wg_w = np.asarray(inputs["wg_w"], dtype=f32)
    wg_b = np.asarray(inputs["wg_b"], dtype=f32)
    ws_w = np.asarray(inputs["ws_w"], dtype=f32)
    ws_b = np.asarray(inputs["ws_b"], dtype=f32)

    x = np.concatenate([embN[n_input], embT[t_input]], axis=1)   # [32, 768]
    xh = np.concatenate([x, h0], axis=1)                          # [32, 1792]
    xhT = np.ascontiguousarray(xh.T)
    c0T = np.ascontiguousarray(c0.T)                              # [1024, 32]
    W_cat = np.concatenate([W_ih, W_hh], axis=1)                  # [4096, 1792]
    b_cat = b_ih + b_hh
    WhT = np.ascontiguousarray(Wh_w.T)
    WmT = np.ascontiguousarray(Wm_w.T)
    attb = np.ascontiguousarray((Wh_b + Wm_b).reshape(HC, 128).T)
    vTm = np.ascontiguousarray(v_w[0].reshape(HC, 128).T)
    ws1T = np.ascontiguousarray(ws_w[0, :H].reshape(HC, 128).T)
    ws2T = np.ascontiguousarray(ws_w[0, H:].reshape(HC, 128).T)
    wsb4 = np.full((BPC, 1), float(ws_b[0]), dtype=f32)
    hparT = np.ascontiguousarray(h_parent.T)
    identm = np.eye(128, dtype=f32)
    ones1 = np.ones((1, 32), dtype=f32)
    wgT_full = np.ascontiguousarray(wg_w.T)                       # [3072, 50003]

    in_maps = []
    for k in range(N_CORES):
        rows = np.concatenate([np.arange(128) + 128 * k + 1024 * g for g in range(4)])
        wlstmT = np.ascontiguousarray(W_cat[rows].T)              # [1792, 512]
        lstmb = np.ascontiguousarray(b_cat[rows].reshape(4, 128).T)
        v0 = VPC * k
        v1 = min(V, v0 + VPC)
        wgT_k = np.zeros((3072, VPC), dtype=f32)
        wgT_k[:, :v1 - v0] = wgT_full[:, v0:v1]
        wgb_k = np.full((1, VPC), NEG, dtype=f32)
        wgb_k[0, :v1 - v0] = wg_b[v0:v1]
        e = enc[BPC * k:BPC * (k + 1)]                            # [4, 128, 1024]
        encT_k = np.ascontiguousarray(
            np.transpose(e, (2, 0, 1)).reshape(H, BPC * L)
        )
        encN_k = np.ascontiguousarray(e.reshape(BPC * L, H))
        maskf_k = np.where(
            mask[BPC * k:BPC * (k + 1)], f32(-1e20), f32(0)
        ).astype(f32).reshape(1, BPC * L)
        sel_k = np.zeros((B, BPC), dtype=f32)
        for j in range(BPC):
            sel_k[BPC * k + j, j] = 1.0
        in_maps.append({
            "wgT": wgT_k, "wgb": wgb_k,
            "wlstmT": wlstmT, "lstmb": lstmb,
            "xhT": xhT, "c0T": np.ascontiguousarray(c0T[128 * k:128 * (k + 1)]),
            "WhT": WhT, "WmT": WmT, "attb": attb,
            "encT": encT_k, "encN": encN_k,
            "vT": vTm, "maskf": maskf_k, "sel": sel_k,
            "ws1T": ws1T, "ws2T": ws2T, "wsb4": wsb4,
            "hparT": hparT, "ident": identm, "ones1": ones1,
        })
    return in_maps


def assemble(results):
    f32 = np.float32
    lse_k = np.stack([results[k]["lse"][:, 0] for k in range(N_CORES)])  # [8, 32]
    m = lse_k.max(axis=0)
    lse_g = m + np.log(np.exp(lse_k - m).sum(axis=0))                    # [32]
    st = np.concatenate([results[k]["st"][:, 0] for k in range(N_CORES)])  # [32]

    logits = np.empty((B, V + L), dtype=f32)
    shift = (st - lse_g).astype(f32)                                     # [32]
    for k in range(N_CORES):
        v0 = VPC * k
        v1 = min(V, v0 + VPC)
        logits[:, v0:v1] = results[k]["z"][:, :v1 - v0] + shift[:, None]
    for k in range(N_CORES):
        logits[:, V:][BPC * k:BPC * (k + 1)] = results[k]["clp"]

    h_new = np.concatenate(
        [results[k]["hnat"] for k in range(N_CORES)], axis=1
    )[None].astype(f32)
    c_new = np.concatenate(
        [results[k]["cT"].T for k in range(N_CORES)], axis=1
    )[None].astype(f32)
    return logits, h_new, c_new


_NC_CACHE = {}


def get_nc(single_core=False):
    key = bool(single_core)
    if key not in _NC_CACHE:
        _NC_CACHE[key] = build_nc(single_core)
    return _NC_CACHE[key]


def kernel(**inputs):
    nc = get_nc()
    in_maps = make_in_maps(inputs)
    res = run_bass_kernel_spmd(nc, in_maps, list(range(N_CORES)))
    return assemble(res.results)


# revision 20
# speedup vs baseline: 1.1919x; 1.1919x over previous
"""Trainium2 Bass kernel for nn_DecoderAttention (LSTM cell + additive attention
+ pointer-generator vocab distribution).

Sharding (8 cores):
  - wg_w (the [50003, 3072] output projection) is row-sharded (vocab-parallel);
    each core streams a [3072, 6272] pre-transposed fp32 shard through the PE
    at full rate (float32r) against the stationary x_catT chunks.
  - The LSTM cell is hidden-sharded: core k computes h_new/c_new for hidden
    dims [128k, 128k+128) for all 32 batches.
  - Attention is batch-sharded: core k handles batches [4k, 4k+4).
  - Two small AllGathers (h_new, context) knit the stages together; the
    weight stream is ordered h_parent-third -> h_new-third -> context-third
    so DMA never waits on the dependency chain.
  - Per-shard log-softmax stats (logsumexp) are computed on device; the final
    cross-shard combine and output concatenation happen on host.
"""

import numpy as np

import concourse.bass as bass
import concourse.bacc as bacc
import concourse.tile as tile
import concourse.mybir as mybir
from concourse.bass_utils import run_bass_kernel_spmd

dt = mybir.dt
F32 = dt.float32
F32R = dt.float32r
AF = mybir.ActivationFunctionType

N_CORES = 8
B, L, H = 32, 128, 1024
EIN = 768           # embN(256) + embT(512)
XH = EIN + H        # 1792 = lstm input + hidden
KC_XH = XH // 128   # 14
V = 50003           # VOUT
VPC = 6272          # padded vocab shard per core (8*6272 = 50176)
BPC = B // N_CORES  # 4 batches per core
HC = 8              # hidden chunks of 128
NEG = -1.0e9        # pad bias for log-softmax masking

# N-strip layout of the vocab shard: 6 strips of 1024 (2 x 512 matmuls each)
# plus one tail strip of 128.
STRIPS = [(g * 1024, 1024, 2) for g in range(6)] + [(6144, 128, 1)]
NCHUNKS = 13  # 12 x 512 + 1 x 128


def _r(ap):
    return ap.bitcast(F32R)


def build_nc(single_core: bool = False):
    """Trace + compile the SPMD program. Returns the compiled Bacc."""
    nc = bacc.Bacc(
        "TRN2",
        target_bir_lowering=False,
        debug=False,
        num_devices=1 if single_core else N_CORES,
    )

    # ---- DRAM I/O (per core) ----
    wgT = nc.dram_tensor("wgT", [3072, VPC], F32R, kind="ExternalInput")
    wgb = nc.dram_tensor("wgb", [1, VPC], F32R, kind="ExternalInput")
    wlstmT = nc.dram_tensor("wlstmT", [XH, 512], F32, kind="ExternalInput")
    lstmb = nc.dram_tensor("lstmb", [128, 4], F32, kind="ExternalInput")
    xhT = nc.dram_tensor("xhT", [XH, 32], F32, kind="ExternalInput")
    c0T = nc.dram_tensor("c0T", [128, 32], F32, kind="ExternalInput")
    WhT = nc.dram_tensor("WhT", [H, H], F32R, kind="ExternalInput")
    WmT = nc.dram_tensor("WmT", [H, H], F32R, kind="ExternalInput")
    attb = nc.dram_tensor("attb", [128, HC], F32, kind="ExternalInput")
    encT = nc.dram_tensor("encT", [H, BPC * L], F32R, kind="ExternalInput")
    vT = nc.dram_tensor("vT", [128, HC], F32, kind="ExternalInput")
    maskf = nc.dram_tensor("maskf", [1, BPC * L], F32, kind="ExternalInput")
    sel = nc.dram_tensor("sel", [B, BPC], F32R, kind="ExternalInput")
    ws1T = nc.dram_tensor("ws1T", [128, HC], F32R, kind="ExternalInput")
    ws2T = nc.dram_tensor("ws2T", [128, HC], F32R, kind="ExternalInput")
    wsb4 = nc.dram_tensor("wsb4", [BPC, 1], F32, kind="ExternalInput")
    hparT = nc.dram_tensor("hparT", [H, 32], F32R, kind="ExternalInput")
    ident = nc.dram_tensor("ident", [128, 128], F32R, kind="ExternalInput")
    ones1 = nc.dram_tensor("ones1", [1, 32], F32R, kind="ExternalInput")
    onesf = nc.dram_tensor("onesf", [1, 128], F32, kind="ExternalInput")

    z_out = nc.dram_tensor("z", [32, VPC], F32, kind="ExternalOutput")
    lse_out = nc.dram_tensor("lse", [32, 1], F32, kind="ExternalOutput")
    hnat_out = nc.dram_tensor("hnat", [32, 128], F32R, kind="ExternalOutput")
    cT_out = nc.dram_tensor("cT", [128, 32], F32, kind="ExternalOutput")
    clp_out = nc.dram_tensor("clp", [BPC, L], F32, kind="ExternalOutput")
    st_out = nc.dram_tensor("st", [BPC, 1], F32, kind="ExternalOutput")

    groups = [[0]] if single_core else [list(range(N_CORES))]

    with tile.TileContext(nc) as tc:
        with (
            tc.tile_pool(name="stream", bufs=11) as streamp,
            tc.tile_pool(name="weights", bufs=8) as wxp,
            tc.tile_pool(name="encp", bufs=1) as encp,
            tc.tile_pool(name="zp", bufs=1) as zp,
            tc.tile_pool(name="smallp", bufs=1) as smallp,
            tc.tile_pool(name="tanhp", bufs=2) as tanhp,
            tc.tile_pool(name="wgbp", bufs=3) as wgbp,
            tc.tile_pool(name="expp", bufs=2) as expp,
            tc.tile_pool(name="zps", bufs=3, space="PSUM") as zpsp,
            tc.tile_pool(name="bigps", bufs=2, space="PSUM") as bigps,
            tc.tile_pool(name="scps", bufs=1, space="PSUM") as scps,
            tc.tile_pool(name="tinyps", bufs=2, space="PSUM") as tinyps,
            tc.tile_pool(name="dram", bufs=1, space="DRAM") as dramp,
        ):
            # ---------- small constant loads ----------
            ident_sb = smallp.tile([128, 128], F32R, tag="ident")
            nc.scalar.dma_start(ident_sb[:], ident[:])
            ones_sb = smallp.tile([1, 32], F32R, tag="ones1")
            nc.scalar.dma_start(ones_sb[:], ones1[:])
            ones128 = smallp.tile([1, 128], F32, tag="onesf")
            nc.scalar.dma_start(ones128[:], onesf[:])
            sel_sb = smallp.tile([B, BPC], F32R, tag="sel")
            nc.scalar.dma_start(sel_sb[:], sel[:])
            lstmb_sb = smallp.tile([128, 4], F32, tag="lstmb")
            nc.scalar.dma_start(lstmb_sb[:], lstmb[:])
            attb_sb = smallp.tile([128, HC], F32, tag="attb")
            nc.scalar.dma_start(attb_sb[:], attb[:])
            vT_sb = smallp.tile([128, HC], F32, tag="vT")
            nc.scalar.dma_start(vT_sb[:], vT[:])
            ws1_sb = smallp.tile([128, HC], F32R, tag="ws1")
            nc.scalar.dma_start(ws1_sb[:], ws1T[:])
            ws2_sb = smallp.tile([128, HC], F32R, tag="ws2")
            nc.scalar.dma_start(ws2_sb[:], ws2T[:])
            wsb_sb = smallp.tile([BPC, 1], F32, tag="wsb")
            nc.scalar.dma_start(wsb_sb[:], wsb4[:])
            maskf_sb = smallp.tile([1, BPC * L], F32, tag="maskf")
            nc.scalar.dma_start(maskf_sb[:], maskf[:])
            xhT_sb = smallp.tile([128, KC_XH, 32], F32, tag="xhT")
            nc.scalar.dma_start(
                xhT_sb[:], xhT[:].rearrange("(c p) b -> p c b", p=128)
            )
            c0T_sb = smallp.tile([128, 32], F32, tag="c0T")
            nc.scalar.dma_start(c0T_sb[:], c0T[:])

            # x_catT stationary chunks: 24 x [128, 32].
            # 0..7 = context (A), 8..15 = h_new (B), 16..23 = h_parent (C)
            xcat = [
                smallp.tile([128, 32], F32R, tag=f"xcat{kc}", name=f"xcat{kc}")
                for kc in range(24)
            ]
            hparT_r = hparT[:].rearrange("(c p) b -> c p b", p=128)
            for hcc in range(HC):
                nc.scalar.dma_start(xcat[16 + hcc][:], hparT_r[hcc])

            # ---------- LSTM (hidden-sharded: this core's 128 h dims) ----------
            gates_ps = tinyps.tile([128, 4, 32], F32, tag="tiny")
            wls = []
            for pi in range(KC_XH // 2):
                wl = streamp.tile([128, 2, 512], F32, tag="wg", name=f"wl{pi}")
                nc.sync.dma_start(
                    wl[:],
                    wlstmT[:].rearrange("(c p) g -> c p g", p=128)
                    [2 * pi:2 * pi + 2].rearrange("c p g -> p c g"),
                )
                wls.append(wl)
            for gc in range(4):
                for kc in range(KC_XH):
                    nc.tensor.matmul(
                        gates_ps[:, gc, :],
                        wls[kc // 2][:, kc % 2, gc * 128:(gc + 1) * 128],
                        xhT_sb[:, kc, :],
                        start=(kc == 0),
                        stop=(kc == KC_XH - 1),
                    )
            ig = smallp.tile([128, 32], F32, tag="ig")
            fg = smallp.tile([128, 32], F32, tag="fg")
            gg = smallp.tile([128, 32], F32, tag="gg")
            og = smallp.tile([128, 32], F32, tag="og")
            nc.scalar.activation(ig[:], gates_ps[:, 0, :], AF.Sigmoid, bias=lstmb_sb[:, 0:1])
            nc.scalar.activation(fg[:], gates_ps[:, 1, :], AF.Sigmoid, bias=lstmb_sb[:, 1:2])
            nc.scalar.activation(gg[:], gates_ps[:, 2, :], AF.Tanh, bias=lstmb_sb[:, 2:3])
            nc.scalar.activation(og[:], gates_ps[:, 3, :], AF.Sigmoid, bias=lstmb_sb[:, 3:4])
            cnew = smallp.tile([128, 32], F32, tag="cnew")
            nc.vector.tensor_mul(cnew[:], fg[:], c0T_sb[:])
            nc.vector.tensor_mul(ig[:], ig[:], gg[:])
            nc.vector.tensor_add(cnew[:], cnew[:], ig[:])
            nc.scalar.dma_start(cT_out[:], cnew[:])
            hnewT = smallp.tile([128, 32], F32R, tag="hnewT")
            nc.scalar.activation(hnewT[:], cnew[:], AF.Tanh)
            nc.vector.tensor_mul(hnewT[:], hnewT[:], og[:])

            # natural layout [32, 128]; gather BOTH layouts so the x_catT B
            # chunks come straight out of the gather with no transposes.
            hn_ps = tinyps.tile([32, 128], F32R, tag="tiny")
            nc.tensor.transpose(hn_ps[:], hnewT[:], ident_sb[:])
            hnat = smallp.tile([32, 128], F32R, tag="hnat")
            nc.vector.tensor_copy(hnat[:], hn_ps[:])
            nc.scalar.dma_start(hnat_out[:], hnat[:])

            # ---------- AllGather 1: h_new (natural + T layouts) ----------
            g1in = dramp.tile([2, 4096], F32R, tag="g1in")
            g1out = dramp.tile([N_CORES, 2, 4096], F32R, tag="g1out")
            nc.scalar.dma_start(g1in[0, :], hnat[:])
            nc.scalar.dma_start(g1in[1, :], hnewT[:])
            if single_core:
                nc.scalar.dma_start(
                    g1out[:], g1in[:].rearrange('(o a) b -> o a b', o=1).broadcast_to([N_CORES, 2, 4096])
                )
            else:
                nc.gpsimd.collective_compute(
                    "AllGather",
                    mybir.AluOpType.bypass,
                    replica_groups=groups,
                    ins=[g1in[:]],
                    outs=[g1out[:]],
                )
            # full h_new, natural layout: [32 batches, (hc, 128)]
            hna = smallp.tile([32, HC, 128], F32R, tag="hna")
            nc.scalar.dma_start(
                hna[:],
                g1out[:, 0, :].rearrange("c (b h) -> b c h", b=32),
            )
            # x_catT B part: direct copy of each core's h_newT chunk
            for hcc in range(HC):
                nc.scalar.dma_start(
                    xcat[8 + hcc][:],
                    g1out[hcc, 1, :].rearrange("(p b) -> p b", p=128),
                )

            # own-batch h in T layout: h_ownT[h, j] for this core's 4 batches
            hown = smallp.tile([128, HC, BPC], F32R, tag="hown")
            for hcc in range(HC):
                ho_ps = tinyps.tile([128, BPC], F32, tag="tiny")
                nc.tensor.matmul(ho_ps[:], hna[:, hcc, :], sel_sb[:])
                nc.vector.tensor_copy(hown[:, hcc, :], ho_ps[:])

            # ---------- attention (batch-sharded: this core's 4 batches) ----------
            encT_sb = encp.tile([128, HC, BPC * L], F32R, tag="encT")
            nc.scalar.dma_start(
                encT_sb[:], encT[:].rearrange("(c p) n -> p c n", p=128)
            )

            # preh[h', b] = sum_h WhT[h, h'] * h_own[h, b]  (+ Wh_b + Wm_b)
            prehb = smallp.tile([128, HC, BPC], F32, tag="prehb")
            WhT_r = WhT[:].rearrange("(ci p) o -> p ci o", p=128)
            for hco in range(HC):
                whb = wxp.tile([128, HC, 128], F32R, tag="wx")
                nc.gpsimd.dma_start(
                    whb[:], WhT_r[:, :, hco * 128:(hco + 1) * 128]
                )
                ph_ps = tinyps.tile([128, BPC], F32, tag="tiny")
                for hci in range(HC):
                    nc.tensor.matmul(
                        ph_ps[:],
                        whb[:, hci, :],
                        hown[:, hci, :],
                        start=(hci == 0),
                        stop=(hci == HC - 1),
                    )
                nc.scalar.activation(
                    prehb[:, hco, :], ph_ps[:], AF.Identity,
                    bias=attb_sb[:, hco:hco + 1],
                )

            # prem + tanh + scores
            sc_ps = scps.tile([1, BPC * L], F32, tag="sc")
            WmT_r = WmT[:].rearrange("(ci p) o -> p ci o", p=128)
            for hco in range(HC):
                wmb = wxp.tile([128, HC, 128], F32R, tag="wx")
                nc.gpsimd.dma_start(
                    wmb[:], WmT_r[:, :, hco * 128:(hco + 1) * 128]
                )
                pm_ps = bigps.tile([128, BPC * L], F32, tag="big")
                for hci in range(HC):
                    nc.tensor.matmul(
                        pm_ps[:],
                        wmb[:, hci, :],
                        encT_sb[:, hci, :],
                        start=(hci == 0),
                        stop=(hci == HC - 1),
                    )
                th = tanhp.tile([128, BPC * L], F32, tag="tanh")
                for b in range(BPC):
                    nc.scalar.activation(
                        th[:, b * L:(b + 1) * L],
                        pm_ps[:, b * L:(b + 1) * L],
                        AF.Tanh,
                        bias=prehb[:, hco, b:b + 1],
                    )
                nc.tensor.matmul(
                    sc_ps[:],
                    vT_sb[:, hco:hco + 1],
                    th[:],
                    start=(hco == 0),
                    stop=(hco == HC - 1),
                )

            scores = smallp.tile([1, BPC * L], F32, tag="scores")
            nc.vector.tensor_add(scores[:], sc_ps[:], maskf_sb[:])
            sc4 = smallp.tile([BPC, L], F32, tag="sc4")
            nc.scalar.dma_start(sc4[:], scores[:])

            # softmax / log-softmax over L per batch
            m4n = smallp.tile([BPC, 1], F32, tag="m4n")
            nc.vector.reduce_max(m4n[:], sc4[:], axis=mybir.AxisListType.X, negate=True)
            exp4 = smallp.tile([BPC, L], F32, tag="exp4")
            sum4 = smallp.tile([BPC, 1], F32, tag="sum4")
            nc.scalar.activation(exp4[:], sc4[:], AF.Exp, bias=m4n[:], accum_out=sum4[:])
            rec4 = smallp.tile([BPC, 1], F32, tag="rec4")
            nc.vector.reciprocal(rec4[:], sum4[:])
            attn4 = smallp.tile([BPC, L], F32, tag="attn4")
            nc.vector.tensor_scalar_mul(attn4[:], exp4[:], rec4[:])
            lns4 = smallp.tile([BPC, 1], F32, tag="lns4")
            nc.scalar.activation(lns4[:], sum4[:], AF.Ln)

            # context in T layout via DVE: broadcast attn over partitions,
            # multiply into encT, reduce over l per batch segment.
            attn_flat = smallp.tile([1, BPC * L], F32, tag="attn_flat")
            nc.scalar.dma_start(attn_flat[:], attn4[:])
            bc_ps = bigps.tile([128, BPC * L], F32, tag="big")
            nc.tensor.matmul(bc_ps[:], ones128[:], attn_flat[:])
            ctxT = smallp.tile([128, HC, BPC], F32R, tag="ctxT")
            for hcc in range(HC):
                prod = tanhp.tile([128, BPC * L], F32, tag="prod")
                nc.vector.tensor_mul(
                    prod[:], encT_sb[:, hcc, :].bitcast(F32), bc_ps[:]
                )
                with nc.allow_low_precision(reason="f32r context (16-bit mantissa) is within kernel tolerance"):
                    nc.vector.reduce_sum(
                        ctxT[:, hcc, :],
                        prod[:].rearrange("p (b l) -> p b l", b=BPC),
                        axis=mybir.AxisListType.X,
                    )

            # ---------- AllGather 2: context in T layout ----------
            g2in = dramp.tile([4096], F32R, tag="g2in")
            g2out = dramp.tile([N_CORES, 4096], F32R, tag="g2out")
            nc.scalar.dma_start(g2in[:], ctxT[:])
            if single_core:
                nc.scalar.dma_start(
                    g2out[:], g2in[:].rearrange('(o a) -> o a', o=1).broadcast_to([N_CORES, 4096])
                )
            else:
                nc.gpsimd.collective_compute(
                    "AllGather",
                    mybir.AluOpType.bypass,
                    replica_groups=groups,
                    ins=[g2in[:]],
                    outs=[g2out[:]],
                )
            # x_catT A chunk hc: [p, 4c+b] <- g2out[c, 32p + 4*hc + b]
            g2r = g2out[:].rearrange("c (p h b) -> p h c b", p=128, h=HC)
            for hcc in range(HC):
                nc.scalar.dma_start(xcat[hcc][:], g2r[:, hcc])

            # ---------- pointer switch s_t (this core's 4 batches) ----------
            p_ps = tinyps.tile([1, BPC], F32, tag="tiny")
            for hcc in range(HC):
                nc.tensor.matmul(
                    p_ps[:], ws1_sb[:, hcc:hcc + 1], ctxT[:, hcc, :],
                    start=(hcc == 0), stop=False,
                )
            for hcc in range(HC):
                nc.tensor.matmul(
                    p_ps[:], ws2_sb[:, hcc:hcc + 1], hown[:, hcc, :],
                    start=False, stop=(hcc == HC - 1),
                )
            p1 = smallp.tile([1, BPC], F32, tag="p1")
            nc.vector.tensor_copy(p1[:], p_ps[:])
            p4 = smallp.tile([BPC, 1], F32, tag="p4")
            nc.scalar.dma_start(p4[:], p1[:])
            nc.vector.tensor_add(p4[:], p4[:], wsb_sb[:])
            # s_t = log sigmoid(p) = -softplus(-p) = -ln(1 + exp(-p))
            e_n = smallp.tile([BPC, 1], F32, tag="e_n")
            nc.scalar.activation(e_n[:], p4[:], AF.Exp, scale=-1.0)
            nc.vector.tensor_scalar_add(e_n[:], e_n[:], 1.0)
            sp_n = smallp.tile([BPC, 1], F32, tag="sp_n")
            nc.scalar.activation(sp_n[:], e_n[:], AF.Ln)
            st4 = smallp.tile([BPC, 1], F32, tag="st4")
            nc.vector.tensor_scalar_mul(st4[:], sp_n[:], -1.0)
            nc.scalar.dma_start(st_out[:], st4[:])
            # log(1 - sigmoid(p)) = -softplus(p) = -ln(1 + exp(p))
            e_p = smallp.tile([BPC, 1], F32, tag="e_p")
            nc.scalar.activation(e_p[:], p4[:], AF.Exp)
            nc.vector.tensor_scalar_add(e_p[:], e_p[:], 1.0)
            sp_p = smallp.tile([BPC, 1], F32, tag="sp_p")
            nc.scalar.activation(sp_p[:], e_p[:], AF.Ln)
            # copy_logp = log_attn + logsig(-p)
            #           = sc4 - max - ln(sum) - softplus(p)
            shift = smallp.tile([BPC, 1], F32, tag="shift")
            nc.vector.tensor_add(shift[:], lns4[:], sp_p[:])
            nc.vector.tensor_sub(shift[:], m4n[:], shift[:])
            clp = smallp.tile([BPC, L], F32, tag="clp")
            nc.vector.tensor_scalar_add(clp[:], sc4[:], shift[:])
            nc.scalar.dma_start(clp_out[:], clp[:])

            # ---------- big vocab matmul: 3 K-phases over the shard ----------
            mparts = smallp.tile([32, NCHUNKS], F32, tag="mparts")
            sparts = smallp.tile([32, NCHUNKS], F32, tag="sparts")
            z_tiles = []
            for j in range(NCHUNKS):
                n = 512 if j < 12 else 128
                z_tiles.append(zp.tile([32, n], F32, tag=f"z{j}", name=f"zt{j}"))

            phases = [
                ("C", list(range(16, 24))),
                ("B", list(range(8, 16))),
                ("A", list(range(0, 8))),
            ]
            for pname, kcs in phases:
                for (off, w, nsub) in STRIPS:
                    ptiles = []
                    for pi in range(4):
                        ka = kcs[2 * pi]
                        s = streamp.tile([128, 2, w], F32R, tag="wg",
                                         name=f"s{pname}{off}{pi}")
                        nc.sync.dma_start(
                            s[:],
                            wgT[ka * 128:(ka + 2) * 128, off:off + w]
                            .rearrange("(c p) n -> p c n", p=128),
                        )
                        ptiles.append(s)
                    for sidx in range(nsub):
                        j = off // 512 + sidx
                        n = 512 if j < 12 else 128
                        so = sidx * 512
                        ps = zpsp.tile([32, n], F32, tag="zps")
                        for i, kc in enumerate(kcs):
                            nc.tensor.matmul(
                                ps[:],
                                xcat[kc][:],
                                ptiles[i // 2][:, i % 2, so:so + n],
                                start=(i == 0),
                                stop=(i == len(kcs) - 1 and pname != "C"),
                            )
                        if pname == "C":
                            wgb_t = wgbp.tile([1, 512], F32R, tag="wgb",
                                              name=f"wgb{j}")
                            nc.scalar.dma_start(
                                wgb_t[:, :n], wgb[0:1, off + so:off + so + n]
                            )
                            nc.tensor.matmul(
                                ps[:], ones_sb[:], wgb_t[:, :n],
                                start=False, stop=True,
                            )
                            nc.vector.tensor_copy(z_tiles[j][:], ps[:])
                        else:
                            nc.vector.tensor_add(z_tiles[j][:], z_tiles[j][:], ps[:])
                        if pname == "A":
                            # chunk-local max + sumexp, combined exactly below
                            nc.vector.reduce_max(
                                mparts[:, j:j + 1], z_tiles[j][:],
                                axis=mybir.AxisListType.X, negate=True,
                            )
                            scr = expp.tile([32, 512], F32, tag="exps",
                                            name=f"scr{j}")
                            nc.scalar.activation(
                                scr[:, :n], z_tiles[j][:], AF.Exp,
                                bias=mparts[:, j:j + 1],
                                accum_out=sparts[:, j:j + 1],
                            )
                            nc.scalar.dma_start(
                                z_out[:, j * 512:j * 512 + n], z_tiles[j][:]
                            )

            # ---------- combine chunk-local stats exactly ----------
            # mparts holds -m_j, sparts holds s_j = sum exp(z_j - m_j).
            # lse = m + ln(sum_j s_j * exp(m_j - m)),  m = max_j m_j
            mneg = smallp.tile([32, 1], F32, tag="mneg")
            # mneg = -m = max_j(-m_j) ... need min! -m = min_j(-m_j)? No:
            # m = max_j m_j  =>  -m = min_j(-m_j) = -max over (m_j).
            # reduce over mparts (-m_j) with max+negate gives -max(-m_j)=min m_j.
            # Instead compute m via negate on the already-negated parts:
            # max_j m_j = max_j(-mparts_j): negate input by scaling in ACT.
            mpos = smallp.tile([32, NCHUNKS], F32, tag="mpos")
            nc.scalar.activation(mpos[:], mparts[:], AF.Identity, scale=-1.0)
            nc.vector.reduce_max(
                mneg[:], mpos[:], axis=mybir.AxisListType.X, negate=True
            )
            # w_j = exp(m_j - m) = exp(mpos_j + mneg)
            wj = smallp.tile([32, NCHUNKS], F32, tag="wj")
            nc.scalar.activation(wj[:], mpos[:], AF.Exp, bias=mneg[:])
            nc.vector.tensor_mul(wj[:], wj[:], sparts[:])
            ssum = smallp.tile([32, 1], F32, tag="ssum")
            nc.vector.reduce_sum(ssum[:], wj[:], axis=mybir.AxisListType.X)
            lns = smallp.tile([32, 1], F32, tag="lns")
            nc.scalar.activation(lns[:], ssum[:], AF.Ln)
            lse = smallp.tile([32, 1], F32, tag="lse")
            nc.vector.tensor_sub(lse[:], lns[:], mneg[:])
            nc.scalar.dma_start(lse_out[:], lse[:])

    nc.compile()
    return nc


# ----------------------------------------------------------------------------
# host-side sharding / unsharding
# ----------------------------------------------------------------------------

def make_in_maps(inputs):
    f32 = np.float32
    n_input = np.asarray(inputs["n_input"]).astype(np.int64)
    t_input = np.asarray(inputs["t_input"]).astype(np.int64)
    h0 = np.asarray(inputs["h0"], dtype=f32)[0]        # [32, 1024]
    c0 = np.asarray(inputs["c0"], dtype=f32)[0]
    enc = np.asarray(inputs["enc_out"], dtype=f32)     # [32, 128, 1024]
    mask = np.asarray(inputs["mask"])
    h_parent = np.asarray(inputs["h_parent"], dtype=f32)
    embN = np.asarray(inputs["embN"], dtype=f32)
    embT = np.asarray(inputs["embT"], dtype=f32)
    W_ih = np.asarray(inputs["W_ih"], dtype=f32)
    W_hh = np.asarray(inputs["W_hh"], dtype=f32)
    b_ih = np.asarray(inputs["b_ih"], dtype=f32)
    b_hh = np.asarray(inputs["b_hh"], dtype=f32)
    Wh_w = np.asarray(inputs["Wh_w"], dtype=f32)
    Wh_b = np.asarray(inputs["Wh_b"], dtype=f32)
    Wm_w = np.asarray(inputs["Wm_w"], dtype=f32)
    Wm_b = np.asarray(inputs["Wm_b"], dtype=f32)
    v_w = np.asarray(inputs["v_w"], dtype=f32)
    wg_w = np.asarray(inputs["wg_w"], dtype=f32)
    wg_b = np.asarray(inputs["wg_b"], dtype=f32)
    ws_w = np.asarray(inputs["ws_w"], dtype=f32)
    ws_b = np.asarray(inputs["ws_b"], dtype=f32)

    x = np.concatenate([embN[n_input], embT[t_input]], axis=1)   # [32, 768]
    xh = np.concatenate([x, h0], axis=1)                          # [32, 1792]
    xhT = np.ascontiguousarray(xh.T)
    c0T = np.ascontiguousarray(c0.T)                              # [1024, 32]
    W_cat = np.concatenate([W_ih, W_hh], axis=1)                  # [4096, 1792]
    b_cat = b_ih + b_hh
    WhT = np.ascontiguousarray(Wh_w.T)
    WmT = np.ascontiguousarray(Wm_w.T)
    attb = np.ascontiguousarray((Wh_b + Wm_b).reshape(HC, 128).T)
    vTm = np.ascontiguousarray(v_w[0].reshape(HC, 128).T)
    ws1T = np.ascontiguousarray(ws_w[0, :H].reshape(HC, 128).T)
    ws2T = np.ascontiguousarray(ws_w[0, H:].reshape(HC, 128).T)
    wsb4 = np.full((BPC, 1), float(ws_b[0]), dtype=f32)
    hparT = np.ascontiguousarray(h_parent.T)
    identm = np.eye(128, dtype=f32)
    ones1 = np.ones((1, 32), dtype=f32)
    onesf = np.ones((1, 128), dtype=f32)
    wgT_full = np.ascontiguousarray(wg_w.T)                       # [3072, 50003]

    in_maps = []
    for k in range(N_CORES):
        rows = np.concatenate([np.arange(128) + 128 * k + 1024 * g for g in range(4)])
        wlstmT = np.ascontiguousarray(W_cat[rows].T)              # [1792, 512]
        lstmb = np.ascontiguousarray(b_cat[rows].reshape(4, 128).T)
        v0 = VPC * k
        v1 = min(V, v0 + VPC)
        wgT_k = np.zeros((3072, VPC), dtype=f32)
        wgT_k[:, :v1 - v0] = wgT_full[:, v0:v1]
        wgb_k = np.full((1, VPC), NEG, dtype=f32)
        wgb_k[0, :v1 - v0] = wg_b[v0:v1]
        e = enc[BPC * k:BPC * (k + 1)]                            # [4, 128, 1024]
        encT_k = np.ascontiguousarray(
            np.transpose(e, (2, 0, 1)).reshape(H, BPC * L)
        )
        maskf_k = np.where(
            mask[BPC * k:BPC * (k + 1)], f32(-1e20), f32(0)
        ).astype(f32).reshape(1, BPC * L)
        sel_k = np.zeros((B, BPC), dtype=f32)
        for j in range(BPC):
            sel_k[BPC * k + j, j] = 1.0
        in_maps.append({
            "wgT": wgT_k, "wgb": wgb_k,
            "wlstmT": wlstmT, "lstmb": lstmb,
            "xhT": xhT, "c0T": np.ascontiguousarray(c0T[128 * k:128 * (k + 1)]),
            "WhT": WhT, "WmT": WmT, "attb": attb,
            "encT": encT_k,
            "vT": vTm, "maskf": maskf_k, "sel": sel_k,
            "ws1T": ws1T, "ws2T": ws2T, "wsb4": wsb4,
            "hparT": hparT, "ident": identm, "ones1": ones1, "onesf": onesf,
        })
    return in_maps


def assemble(results):
    f32 = np.float32
    lse_k = np.stack([results[k]["lse"][:, 0] for k in range(N_CORES)])  # [8, 32]
    m = lse_k.max(axis=0)
    lse_g = m + np.log(np.exp(lse_k - m).sum(axis=0))                    # [32]
    st = np.concatenate([results[k]["st"][:, 0] for k in range(N_CORES)])  # [32]

    logits = np.empty((B, V + L), dtype=f32)
    shift = (st - lse_g).astype(f32)                                     # [32]
    for k in range(N_CORES):
        v0 = VPC * k
        v1 = min(V, v0 + VPC)
        logits[:, v0:v1] = results[k]["z"][:, :v1 - v0] + shift[:, None]
    for k in range(N_CORES):
        logits[:, V:][BPC * k:BPC * (k + 1)] = results[k]["clp"]

    h_new = np.concatenate(
        [results[k]["hnat"] for k in range(N_CORES)], axis=1
    )[None].astype(f32)
    c_new = np.concatenate(
        [results[k]["cT"].T for k in range(N_CORES)], axis=1
    )[None].astype(f32)
    return logits, h_new, c_new


_NC_CACHE = {}


def get_nc(single_core=False):
    key = bool(single_core)
    if key not in _NC_CACHE:
        _NC_CACHE[key] = build_nc(single_core)
    return _NC_CACHE[key]


def kernel(**inputs):
    nc = get_nc()
    in_maps = make_in_maps(inputs)
    res = run_bass_kernel_spmd(nc, in_maps, list(range(N_CORES)))
    return assemble(res.results)
